# revision 69
# baseline (speedup 1.0000x reference)
"""Trainium2 Bass kernel for nn_NeuroKernel_56590489092176.

Math (reference):
    P = N(N+1)/2 upper-tri pairs (x[i], x[j]), j >= i, N = 2048
    h  = sigmoid(pairs @ W1.T + b1)     # [P, 128]
    h  = relu(h @ W2.T + b2)            # [P, 32]
    v  = h @ W3.T + b3                  # [P]
    K  = zeros(N, N); K[triu] = v
    out = K.T @ K

Key identity: v(i, j) = g(x_i, x_j) for a fixed smooth g: R^2 -> R, so
instead of running the MLP on all 2.1M pairs, fit a tensor-product cubic
B-spline to g on a G x G grid (host, ~16k MLP evals) and evaluate it on
the device as a rank-G bilinear form:

    V = A @ C @ A.T          A[i, p] = B_p(x_i)  (G = 128 basis funcs)
    K = triu_mask * V

Quantized end-to-end (bf16 interp, fp16 K/GEMM) this reproduces the
reference to ~9e-4 max-rel -- versus the 2e-2 gate.

Distribution (8 cores, contraction-sharded GEMM):
    Core c owns two 128-row strips of K: TOP = rows [128c, 128c+128)
    (all < 1024) and BOT = rows [2048-128(c+1), 2048-128c) (all >=
    1024).  The mirrored pairing keeps per-core work uniform (SPMD: all
    cores run ONE program), and the uniform row bounds expose structure:
      - TOP cols [1024:2048) are all-valid  -> plain ACT copy, no mask
      - BOT cols [0:1024)    are all-zero   -> never computed or read
      - BOT contributes only to out tiles a >= 8 (rows/cols >= 1024)
        -> the GEMM drops from 34816 to 22016 PE cycles

    Single NEFF per core:
      interp  TensorE bf16: V chunks = A_strip @ (C A^T), contraction
              G = 128, one matmul per 512-col chunk (3 of 4 chunks)
      mask    DVE: fused PSUM->SBUF mask-multiply -> fp16 for the two
              mixed chunks (TOP ch0, BOT ch1); masks are per-core fp8
      gemm    TensorE fp16: C_c = TOP^T TOP + [a>=8] BOT^T BOT,
              128-block upper triangle only (C symmetric; host
              mirrors); odd row-tiles skip their first 128 cols
      copy    PSUM->SBUF fp16, greedy DVE/ACT balance
      out     cpart [2048, 2048] fp16 upper-tri; pair 0 ships
              per-1024-chunk on SP (starts the DMA stream ~1us
              earlier), the rest per-row-tile alternating the
              SP-HWDGE and Pool-SWDGE queues; TOP/BOT mask halves
              arrive as separate DMAs so the front chain unblocks
              at the TOP half

    Host: C = sum_c cpart_c (fp32), zero unwritten blocks, mirror, halve
    the double-counted 16 diagonal 128-blocks.

Self-contained: hardcodes all shapes; only needs /opt/trn_rl_repo.
"""

import sys

if "/opt/trn_rl_repo" not in sys.path:
    sys.path.insert(0, "/opt/trn_rl_repo")

import numpy as np

import concourse.bass as bass
import concourse.bacc as bacc
import concourse.mybir as mybir
import concourse.tile as tile
from concourse.bass_utils import run_bass_kernel_spmd

N = 2048
NCORES = 8
G = 128              # spline basis size == one matmul contraction chunk
F32 = mybir.dt.float32
BF16 = mybir.dt.bfloat16
F16 = mybir.dt.float16
ALU = mybir.AluOpType


# ------------------------------------------------- host: B-spline machinery

def _interp_knots(grid, k=3):
    """Knot vector for spline interpolation at `grid` sites (not-a-knot
    style: first/last interior sites dropped; matches scipy s=0)."""
    return np.concatenate([[grid[0]] * (k + 1), grid[2:-2], [grid[-1]] * (k + 1)])


def _bspline_design(xs, t, k=3):
    """Dense design matrix [len(xs), len(t)-k-1] of degree-k B-splines
    (de Boor's basis-funs recursion, vectorized over xs)."""
    xs = np.asarray(xs, np.float64)
    n = len(t) - k - 1
    m = len(xs)
    span = np.clip(np.searchsorted(t, xs, side="right") - 1, k, n - 1)
    Nb = np.zeros((m, k + 1))
    Nb[:, 0] = 1.0
    left = np.zeros((m, k + 1))
    right = np.zeros((m, k + 1))
    for j in range(1, k + 1):
        left[:, j] = xs - t[span + 1 - j]
        right[:, j] = t[span + j] - xs
        saved = np.zeros(m)
        for r in range(j):
            temp = Nb[:, r] / (right[:, r + 1] + left[:, j - r])
            Nb[:, r] = saved + right[:, r + 1] * temp
            saved = left[:, j - r] * temp
        Nb[:, j] = saved
    A = np.zeros((m, n))
    rows = np.repeat(np.arange(m), k + 1)
    cols = (span[:, None] - k + np.arange(k + 1)[None, :]).ravel()
    A[rows, cols.clip(0, n - 1)] = Nb.ravel()
    return A


def _core_strip_rows(c):
    """Core c's two 128-row strips: TOP = K rows [128c, 128c+128) (all
    < 1024) and BOT = rows [2048-128(c+1), 2048-128c) (all >= 1024).
    Mirrored pairing keeps per-core work uniform, and the uniform row
    bounds make TOP's cols [1024:2048) all-valid (plain copy, no mask)
    and BOT's cols [0:1024) all-zero (never computed or read)."""
    return [
        list(range(128 * c, 128 * c + 128)),
        list(range(2048 - 128 * (c + 1), 2048 - 128 * c)),
    ]


def _host_prep(x, W1, b1, W2, b2, W3, b3):
    import ml_dtypes

    bf16 = ml_dtypes.bfloat16
    x64 = np.asarray(x, np.float64)
    lo, hi = x64.min() - 1e-6, x64.max() + 1e-6
    grid = np.linspace(lo, hi, G)

    # exact MLP on the G x G grid of pair values (host, f64)
    gi = np.broadcast_to(grid[:, None], (G, G)).ravel()
    gj = np.broadcast_to(grid[None, :], (G, G)).ravel()
    P = np.stack([gi, gj], axis=-1)
    h = 1.0 / (1.0 + np.exp(-(P @ W1.astype(np.float64).T + b1.astype(np.float64))))
    h = np.maximum(h @ W2.astype(np.float64).T + b2.astype(np.float64), 0.0)
    Gv = (h @ W3.astype(np.float64).T + b3.astype(np.float64))[:, 0].reshape(G, G)

    # spline coefficients and design matrix
    t = _interp_knots(grid)
    M = _bspline_design(grid, t)
    C = np.linalg.solve(M, np.linalg.solve(M, Gv).T).T
    A = _bspline_design(x64, t)                     # [N, G]
    Th = np.ascontiguousarray((C @ A.T).astype(bf16))   # [G, N]

    in_maps = []
    corr = np.zeros((N, N), np.float64)
    T32 = (C @ A.T).astype(np.float32)
    for c in range(NCORES):
        strips = _core_strip_rows(c)
        at = np.zeros((G, 256), np.float64)
        mk = np.zeros((128, N), np.float64)
        for s, rows in enumerate(strips):
            at[:, 128 * s : 128 * s + 128] = A[rows].T
        # BOT mask only ([:, 1024:2048] = BOT rows vs cols [1024:2048));
        # TOP ch0 ships UNMASKED: zeroed th columns handle the all-zero
        # region, and the staircase spill E is subtracted on the host.
        bot = strips[1]
        full = np.arange(N)[None, :] >= np.asarray(bot)[:, None]
        mk[:, 1024:2048] = full[:, 1024:2048]
        thc = np.asarray(Th).copy()
        thc[:, 0 : 128 * c] = 0
        # correction: device TOP strip = K_top + E with
        # E[r, j] = V[r, j] for 128c <= j < r (strictly-lower staircase)
        Vt = A[strips[0]].astype(np.float32) @ T32            # [128, N]
        rr = np.asarray(strips[0])
        Kt = np.where(np.arange(N)[None, :] >= rr[:, None], Vt, 0.0)
        E = np.zeros((128, N), np.float32)
        blk = slice(128 * c, 128 * c + 128)
        E[:, blk] = np.where(
            (np.arange(128 * c, 128 * c + 128)[None, :] < rr[:, None]), Vt[:, blk], 0.0
        )
        KTE = Kt.T @ E[:, blk]                                # [N, 128]
        corr[:, blk] += KTE
        corr[blk, :] += KTE.T
        corr[blk, blk] += E[:, blk].T @ E[:, blk]
        in_maps.append(
            {"at": at.astype(bf16), "th": thc, "mk": mk.astype(ml_dtypes.float8_e4m3)}
        )
    return in_maps, corr


# ------------------------------------------------------------- the NEFF

def build_nc():
    nc = bacc.Bacc("TRN2", target_bir_lowering=False, debug=False)

    F8 = mybir.dt.float8e4
    atd = nc.dram_tensor("at", [G, 256], BF16, kind="ExternalInput")
    thd = nc.dram_tensor("th", [G, N], BF16, kind="ExternalInput")
    mkd = nc.dram_tensor("mk", [128, N], F8, kind="ExternalInput")
    cpd = nc.dram_tensor("cpart", [N, N], F16, kind="ExternalOutput")

    with tile.TileContext(nc) as tc:
        with (
            tc.tile_pool(name="consts", bufs=1) as consts,
            tc.tile_pool(name="csbp", bufs=8) as csbp,
            tc.tile_pool(name="pp", bufs=4, space="PSUM") as pp,
        ):
            # input DMAs in per-half tiles so consumers unblock early;
            # transfer order on the (serial) DMA engines:
            # th0, at, mk0, th1, mk1
            th0 = consts.tile([G, 1024], BF16, name="th0")
            nc.sync.dma_start(th0[:], thd.ap()[:, 0:1024])
            atsb = consts.tile([G, 256], BF16)
            nc.gpsimd.dma_start(atsb[:], atd.ap())
            mkB = consts.tile([128, 1024], F8, name="mkB")
            nc.scalar.dma_start(mkB[:], mkd.ap()[:, 1024:2048])
            th1 = consts.tile([G, 1024], BF16, name="th1")
            nc.sync.dma_start(th1[:], thd.ap()[:, 1024:2048])
            th_t = [th0, th1]

            # ramp the PE p-state while input DMAs are in flight
            warm = consts.tile([128, 512], F16, tag="warm")
            nc.vector.memset(warm[:], 0.0)
            wp = pp.tile([128, 1024], F32, name="cps")
            for _ in range(4):
                nc.tensor.matmul(
                    wp[:, 0:512], lhsT=warm[:, 0:128], rhs=warm[:, 0:512],
                    start=True, stop=True, skip_group_check=True,
                )

            # greedy DVE/ACT copy assignment by predicted engine busy-ns
            eng_busy = [0.0, 0.0]  # DVE, ACT

            forced = [0, 0, 1]  # TOP-ch0, TOP-ch1 -> DVE; first GEMM copy -> ACT

            def copy_out(dst, src, cols):
                cost = (cols * 1.042 + 290, cols * 0.833 + 390)
                if forced:
                    e = forced.pop(0)
                else:
                    e = 0 if eng_busy[0] + cost[0] <= eng_busy[1] + cost[1] else 1
                eng_busy[e] += cost[e]
                if e == 0:
                    nc.vector.tensor_copy(dst, src)
                else:
                    nc.scalar.copy(dst, src)

            # interp: TOP ch0 = DVE fused mask-copy; TOP ch1 is all-valid
            # (plain ACT copy, no mask); BOT ch0 is all-zero and never read
            # (no matmul, no store); BOT ch1 = DVE fused mask-copy.
            strips = [
                consts.tile([128, N], F16, name=f"s{s}", tag=f"s{s}") for s in range(2)
            ]

            def interp_chunk(s, ch):
                ps = pp.tile([128, 1024], F32, name="cps")
                for q in range(2):
                    nc.tensor.matmul(
                        ps[:, 512 * q : 512 * q + 512],
                        lhsT=atsb[:, 128 * s : 128 * s + 128],
                        rhs=th_t[ch][:, 512 * q : 512 * q + 512],
                        start=True, stop=True, skip_group_check=True,
                    )
                return ps

            ps = interp_chunk(0, 0)
            copy_out(strips[0][:, 0:1024], ps[:], 1024)
            ps = interp_chunk(0, 1)
            copy_out(strips[0][:, 1024:2048], ps[:], 1024)
            ps = interp_chunk(1, 1)
            nc.vector.scalar_tensor_tensor(
                strips[1][:, 1024:2048], ps[:], 1.0, mkB[:],
                op0=ALU.mult, op1=ALU.mult,
            )
            eng_busy[0] += 1024 * 1.042 + 290

            # GEMM: C_c = S0^T S0 + S1^T S1, upper 256-block-triangle.
            # 2048-col PSUM tiles (4 banks); ONE wide copy per a-tile.
            # a-pairs (2k, 2k+1) share a col range: both land in one csb
            # tile and ship as a single merged DMA.
            di = 0
            for k in range(8):
                c0 = 256 * k
                w = 2048 - c0
                if k > 0:
                    csb = csbp.tile([128, 2 * N], F16, name="csb")
                for half in range(2):
                    a = 2 * k + half
                    # odd halves skip their first 128 cols: block
                    # (2k+1, 2k) is the mirror of (2k, 2k+1)
                    h0 = c0 + 128 * half
                    wh = 2048 - h0
                    off = 0
                    while off < wh:
                        cw = min(1024, wh - off)
                        cc0 = h0 + off
                        cps = pp.tile([128, 1024], F32, name="cps")
                        # BOT rows are all >= 1024: zero for out rows < 1024
                        # and for cols < 1024, so s=1 only runs for a >= 8
                        # (where all cols are >= 1024 too)
                        ss = [0] if a < 8 else [0, 1]
                        so = 0
                        while so < cw:  # <=512-col matmuls (one bank each)
                            sw = min(512, cw - so)
                            for s in ss:
                                nc.tensor.matmul(
                                    cps[:, so : so + sw],
                                    lhsT=strips[s][:, 128 * a : 128 * a + 128],
                                    rhs=strips[s][:, cc0 + so : cc0 + so + sw],
                                    start=(s == 0),
                                    stop=(s == ss[-1]),
                                    skip_group_check=True,
                                )
                            so += sw
                        if k == 0:
                            # first pair ships per-chunk: the out-DMA
                            # stream starts ~1us earlier
                            csbq = csbp.tile([128, 1024], F16, name="csbq")
                            copy_out(csbq[:, 0:cw], cps[:, 0:cw], cw)
                            nc.sync.dma_start(
                                cpd.ap()[128 * a : 128 * a + 128, cc0 : cc0 + cw],
                                csbq[:, 0:cw],
                            )
                        else:
                            copy_out(
                                csb[:, half * w + off : half * w + off + cw],
                                cps[:, 0:cw],
                                cw,
                            )
                        off += cw
                    if k > 0:
                        (nc.sync if di % 2 == 0 else nc.gpsimd).dma_start(
                            cpd.ap()[128 * a : 128 * a + 128, h0:2048],
                            csb[:, half * w : half * w + wh],
                        )
                        di += 1

    nc.compile()
    return nc


_NC = None


def _get_nc():
    global _NC
    if _NC is None:
        _NC = build_nc()
    return _NC


def _get_ncs():
    return [_get_nc()]


def kernel(x, W1, b1, W2, b2, W3, b3):
    in_maps, corr = _host_prep(
        np.asarray(x), np.asarray(W1), np.asarray(b1), np.asarray(W2),
        np.asarray(b2), np.asarray(W3), np.asarray(b3),
    )
    res = run_bass_kernel_spmd(_get_nc(), in_maps, core_ids=list(range(NCORES)))
    out = np.zeros((N, N), np.float32)
    for c in range(NCORES):
        out += res.results[c]["cpart"].astype(np.float32)
    # only the upper 256-block-triangle was computed; zero the rest,
    # mirror, and halve the double-counted diagonal 256-blocks
    for bi in range(8):
        out[256 * bi : 256 * (bi + 1), : 256 * bi] = 0.0
        # odd halves skipped their first 128 cols (mirrored below)
        out[256 * bi + 128 : 256 * bi + 256, 256 * bi : 256 * bi + 128] = 0.0
    out = out + out.T
    for bi in range(16):
        sl = slice(128 * bi, 128 * (bi + 1))
        out[sl, sl] *= 0.5
    return out - corr.astype(np.float32)


# revision 70
# speedup vs baseline: 1.0633x; 1.0633x over previous
"""Trainium2 Bass kernel for nn_NeuroKernel_56590489092176.

Math (reference):
    P = N(N+1)/2 upper-tri pairs (x[i], x[j]), j >= i, N = 2048
    h  = sigmoid(pairs @ W1.T + b1)     # [P, 128]
    h  = relu(h @ W2.T + b2)            # [P, 32]
    v  = h @ W3.T + b3                  # [P]
    K  = zeros(N, N); K[triu] = v
    out = K.T @ K

Key identity: v(i, j) = g(x_i, x_j) for a fixed smooth g: R^2 -> R, so
instead of running the MLP on all 2.1M pairs, fit a tensor-product cubic
B-spline to g on a G x G grid (host, ~16k MLP evals) and evaluate it on
the device as a rank-G bilinear form:

    V = A @ C @ A.T          A[i, p] = B_p(x_i)  (G = 128 basis funcs)
    K = triu_mask * V

Quantized end-to-end (bf16 interp, fp16 K/GEMM) this reproduces the
reference to ~9e-4 max-rel -- versus the 2e-2 gate.

Distribution (8 cores, contraction-sharded GEMM):
    Core c owns two 128-row strips of K: TOP = rows [128c, 128c+128)
    (all < 1024) and BOT = rows [2048-128(c+1), 2048-128c) (all >=
    1024).  The mirrored pairing keeps per-core work uniform (SPMD: all
    cores run ONE program), and the uniform row bounds expose structure:
      - TOP cols [1024:2048) are all-valid  -> plain ACT copy, no mask
      - BOT cols [0:1024)    are all-zero   -> never computed or read
      - BOT contributes only to out tiles a >= 8 (rows/cols >= 1024)
        -> the GEMM drops from 34816 to 22016 PE cycles

    Single NEFF per core:
      interp  TensorE bf16: V chunks = A_strip @ (C A^T), contraction
              G = 128, one matmul per 512-col chunk (3 of 4 chunks)
      mask    DVE: fused PSUM->SBUF mask-multiply -> fp16 for the two
              mixed chunks (TOP ch0, BOT ch1); masks are per-core fp8
      gemm    TensorE fp16: C_c = TOP^T TOP + [a>=8] BOT^T BOT,
              128-block upper triangle only (C symmetric; host
              mirrors); odd row-tiles skip their first 128 cols
      copy    PSUM->SBUF fp16, greedy DVE/ACT balance
      out     cpart [2048, 2048] fp16 upper-tri; pair 0 ships
              per-1024-chunk on SP (starts the DMA stream ~1us
              earlier), the rest per-row-tile alternating the
              SP-HWDGE and Pool-SWDGE queues; TOP/BOT mask halves
              arrive as separate DMAs so the front chain unblocks
              at the TOP half

    Host: C = sum_c cpart_c (fp32), zero unwritten blocks, mirror, halve
    the double-counted 16 diagonal 128-blocks.

Self-contained: hardcodes all shapes; only needs /opt/trn_rl_repo.
"""

import sys

if "/opt/trn_rl_repo" not in sys.path:
    sys.path.insert(0, "/opt/trn_rl_repo")

import numpy as np

import concourse.bass as bass
import concourse.bacc as bacc
import concourse.mybir as mybir
import concourse.tile as tile
from concourse.bass_utils import run_bass_kernel_spmd

N = 2048
NCORES = 8
G = 128              # spline basis size == one matmul contraction chunk
F32 = mybir.dt.float32
BF16 = mybir.dt.bfloat16
F16 = mybir.dt.float16
ALU = mybir.AluOpType


# ------------------------------------------------- host: B-spline machinery

def _interp_knots(grid, k=3):
    """Knot vector for spline interpolation at `grid` sites (not-a-knot
    style: first/last interior sites dropped; matches scipy s=0)."""
    return np.concatenate([[grid[0]] * (k + 1), grid[2:-2], [grid[-1]] * (k + 1)])


def _bspline_design(xs, t, k=3):
    """Dense design matrix [len(xs), len(t)-k-1] of degree-k B-splines
    (de Boor's basis-funs recursion, vectorized over xs)."""
    xs = np.asarray(xs, np.float64)
    n = len(t) - k - 1
    m = len(xs)
    span = np.clip(np.searchsorted(t, xs, side="right") - 1, k, n - 1)
    Nb = np.zeros((m, k + 1))
    Nb[:, 0] = 1.0
    left = np.zeros((m, k + 1))
    right = np.zeros((m, k + 1))
    for j in range(1, k + 1):
        left[:, j] = xs - t[span + 1 - j]
        right[:, j] = t[span + j] - xs
        saved = np.zeros(m)
        for r in range(j):
            temp = Nb[:, r] / (right[:, r + 1] + left[:, j - r])
            Nb[:, r] = saved + right[:, r + 1] * temp
            saved = left[:, j - r] * temp
        Nb[:, j] = saved
    A = np.zeros((m, n))
    rows = np.repeat(np.arange(m), k + 1)
    cols = (span[:, None] - k + np.arange(k + 1)[None, :]).ravel()
    A[rows, cols.clip(0, n - 1)] = Nb.ravel()
    return A


def _core_strip_rows(c):
    """Core c's two 128-row strips: TOP = K rows [128c, 128c+128) (all
    < 1024) and BOT = rows [2048-128(c+1), 2048-128c) (all >= 1024).
    Mirrored pairing keeps per-core work uniform, and the uniform row
    bounds make TOP's cols [1024:2048) all-valid (plain copy, no mask)
    and BOT's cols [0:1024) all-zero (never computed or read)."""
    return [
        list(range(128 * c, 128 * c + 128)),
        list(range(2048 - 128 * (c + 1), 2048 - 128 * c)),
    ]


def _host_prep(x, W1, b1, W2, b2, W3, b3):
    import ml_dtypes

    bf16 = ml_dtypes.bfloat16
    x64 = np.asarray(x, np.float64)
    lo, hi = x64.min() - 1e-6, x64.max() + 1e-6
    grid = np.linspace(lo, hi, G)

    # exact MLP on the G x G grid of pair values (host, f64)
    gi = np.broadcast_to(grid[:, None], (G, G)).ravel()
    gj = np.broadcast_to(grid[None, :], (G, G)).ravel()
    P = np.stack([gi, gj], axis=-1)
    h = 1.0 / (1.0 + np.exp(-(P @ W1.astype(np.float64).T + b1.astype(np.float64))))
    h = np.maximum(h @ W2.astype(np.float64).T + b2.astype(np.float64), 0.0)
    Gv = (h @ W3.astype(np.float64).T + b3.astype(np.float64))[:, 0].reshape(G, G)

    # spline coefficients and design matrix
    t = _interp_knots(grid)
    M = _bspline_design(grid, t)
    C = np.linalg.solve(M, np.linalg.solve(M, Gv).T).T
    A = _bspline_design(x64, t)                     # [N, G]
    Th = np.ascontiguousarray((C @ A.T).astype(bf16))   # [G, N]

    in_maps = []
    for c in range(NCORES):
        strips = _core_strip_rows(c)
        at = np.zeros((G, 256), np.float64)
        mk = np.zeros((128, N), np.float64)
        # mask tile [128, 2048]: [:, 0:1024] = TOP rows vs cols [0:1024);
        # [:, 1024:2048] = BOT rows vs cols [1024:2048)
        for s, rows in enumerate(strips):
            at[:, 128 * s : 128 * s + 128] = A[rows].T
            full = np.arange(N)[None, :] >= np.asarray(rows)[:, None]
            mk[:, 1024 * s : 1024 * s + 1024] = full[:, 1024 * s : 1024 * s + 1024]
        in_maps.append(
            {"at": at.astype(bf16), "th": Th, "mk": mk.astype(ml_dtypes.float8_e4m3)}
        )
    return in_maps


# ------------------------------------------------------------- the NEFF

def build_nc():
    nc = bacc.Bacc("TRN2", target_bir_lowering=False, debug=False)

    F8 = mybir.dt.float8e4
    atd = nc.dram_tensor("at", [G, 256], BF16, kind="ExternalInput")
    thd = nc.dram_tensor("th", [G, N], BF16, kind="ExternalInput")
    mkd = nc.dram_tensor("mk", [128, N], F8, kind="ExternalInput")
    cpd = nc.dram_tensor("cpart", [N, N], F16, kind="ExternalOutput")

    with tile.TileContext(nc) as tc:
        with (
            tc.tile_pool(name="consts", bufs=1) as consts,
            tc.tile_pool(name="csbp", bufs=8) as csbp,
            tc.tile_pool(name="pp", bufs=4, space="PSUM") as pp,
        ):
            # input DMAs in per-half tiles so consumers unblock early;
            # transfer order on the (serial) DMA engines:
            # th0, at, mk0, th1, mk1
            mkT = consts.tile([128, 1024], F8, name="mkT")
            nc.scalar.dma_start(mkT[:], mkd.ap()[:, 0:1024])
            th0 = consts.tile([G, 1024], BF16, name="th0")
            nc.sync.dma_start(th0[:], thd.ap()[:, 0:1024])
            atsb = consts.tile([G, 256], BF16)
            nc.gpsimd.dma_start(atsb[:], atd.ap())
            mkB = consts.tile([128, 1024], F8, name="mkB")
            nc.scalar.dma_start(mkB[:], mkd.ap()[:, 1024:2048])
            th1 = consts.tile([G, 1024], BF16, name="th1")
            nc.sync.dma_start(th1[:], thd.ap()[:, 1024:2048])
            th_t = [th0, th1]

            # ramp the PE p-state while input DMAs are in flight
            warm = consts.tile([128, 512], F16, tag="warm")
            nc.vector.memset(warm[:], 0.0)
            wp = pp.tile([128, 1024], F32, name="cps")
            for _ in range(4):
                nc.tensor.matmul(
                    wp[:, 0:512], lhsT=warm[:, 0:128], rhs=warm[:, 0:512],
                    start=True, stop=True, skip_group_check=True,
                )

            # greedy DVE/ACT copy assignment by predicted engine busy-ns
            eng_busy = [0.0, 0.0]  # DVE, ACT

            forced = [1, 0]  # first two GEMM copies: ACT then DVE

            def copy_out(dst, src, cols):
                cost = (cols * 1.042 + 290, cols * 0.833 + 390)
                if forced:
                    e = forced.pop(0)
                else:
                    e = 0 if eng_busy[0] + cost[0] <= eng_busy[1] + cost[1] else 1
                eng_busy[e] += cost[e]
                if e == 0:
                    nc.vector.tensor_copy(dst, src)
                else:
                    nc.scalar.copy(dst, src)

            # interp: TOP ch0 = DVE fused mask-copy; TOP ch1 is all-valid
            # (plain ACT copy, no mask); BOT ch0 is all-zero and never read
            # (no matmul, no store); BOT ch1 = DVE fused mask-copy.
            strips = [
                consts.tile([128, N], F16, name=f"s{s}", tag=f"s{s}") for s in range(2)
            ]

            def interp_chunk(s, ch):
                ps = pp.tile([128, 1024], F32, name="cps")
                for q in range(2):
                    nc.tensor.matmul(
                        ps[:, 512 * q : 512 * q + 512],
                        lhsT=atsb[:, 128 * s : 128 * s + 128],
                        rhs=th_t[ch][:, 512 * q : 512 * q + 512],
                        start=True, stop=True, skip_group_check=True,
                    )
                return ps

            ps = interp_chunk(0, 0)
            nc.vector.scalar_tensor_tensor(
                strips[0][:, 0:1024], ps[:], 1.0, mkT[:],
                op0=ALU.mult, op1=ALU.mult,
            )
            eng_busy[0] += 1024 * 1.042 + 290
            ps = interp_chunk(0, 1)
            nc.scalar.copy(strips[0][:, 1024:2048], ps[:])
            eng_busy[1] += 1024 * 0.833 + 390
            ps = interp_chunk(1, 1)
            nc.vector.scalar_tensor_tensor(
                strips[1][:, 1024:2048], ps[:], 1.0, mkB[:],
                op0=ALU.mult, op1=ALU.mult,
            )
            eng_busy[0] += 1024 * 1.042 + 290

            # GEMM: C_c = S0^T S0 + S1^T S1, upper 256-block-triangle.
            # 2048-col PSUM tiles (4 banks); ONE wide copy per a-tile.
            # a-pairs (2k, 2k+1) share a col range: both land in one csb
            # tile and ship as a single merged DMA.
            di = 0
            for k in range(8):
                c0 = 256 * k
                w = 2048 - c0
                if k > 0:
                    csb = csbp.tile([128, 2 * N], F16, name="csb")
                for half in range(2):
                    a = 2 * k + half
                    # odd halves skip their first 128 cols: block
                    # (2k+1, 2k) is the mirror of (2k, 2k+1)
                    h0 = c0 + 128 * half
                    wh = 2048 - h0
                    off = 0
                    while off < wh:
                        cw = min(1024, wh - off)
                        cc0 = h0 + off
                        cps = pp.tile([128, 1024], F32, name="cps")
                        # BOT rows are all >= 1024: zero for out rows < 1024
                        # and for cols < 1024, so s=1 only runs for a >= 8
                        # (where all cols are >= 1024 too)
                        ss = [0] if a < 8 else [0, 1]
                        so = 0
                        while so < cw:  # <=512-col matmuls (one bank each)
                            sw = min(512, cw - so)
                            for s in ss:
                                nc.tensor.matmul(
                                    cps[:, so : so + sw],
                                    lhsT=strips[s][:, 128 * a : 128 * a + 128],
                                    rhs=strips[s][:, cc0 + so : cc0 + so + sw],
                                    start=(s == 0),
                                    stop=(s == ss[-1]),
                                    skip_group_check=True,
                                )
                            so += sw
                        if k == 0:
                            # first pair ships per-chunk: the out-DMA
                            # stream starts ~1us earlier
                            csbq = csbp.tile([128, 1024], F16, name="csbq")
                            copy_out(csbq[:, 0:cw], cps[:, 0:cw], cw)
                            nc.sync.dma_start(
                                cpd.ap()[128 * a : 128 * a + 128, cc0 : cc0 + cw],
                                csbq[:, 0:cw],
                            )
                        else:
                            copy_out(
                                csb[:, half * w + off : half * w + off + cw],
                                cps[:, 0:cw],
                                cw,
                            )
                        off += cw
                    if k > 0:
                        (nc.sync if di % 2 == 0 else nc.gpsimd).dma_start(
                            cpd.ap()[128 * a : 128 * a + 128, h0:2048],
                            csb[:, half * w : half * w + wh],
                        )
                        di += 1

    nc.compile()
    return nc


_NC = None


def _get_nc():
    global _NC
    if _NC is None:
        _NC = build_nc()
    return _NC


def _get_ncs():
    return [_get_nc()]


def kernel(x, W1, b1, W2, b2, W3, b3):
    in_maps = _host_prep(
        np.asarray(x), np.asarray(W1), np.asarray(b1), np.asarray(W2),
        np.asarray(b2), np.asarray(W3), np.asarray(b3),
    )
    res = run_bass_kernel_spmd(_get_nc(), in_maps, core_ids=list(range(NCORES)))
    out = np.zeros((N, N), np.float32)
    for c in range(NCORES):
        out += res.results[c]["cpart"].astype(np.float32)
    # only the upper 256-block-triangle was computed; zero the rest,
    # mirror, and halve the double-counted diagonal 256-blocks
    for bi in range(8):
        out[256 * bi : 256 * (bi + 1), : 256 * bi] = 0.0
        # odd halves skipped their first 128 cols (mirrored below)
        out[256 * bi + 128 : 256 * bi + 256, 256 * bi : 256 * bi + 128] = 0.0
    out = out + out.T
    for bi in range(16):
        sl = slice(128 * bi, 128 * (bi + 1))
        out[sl, sl] *= 0.5
    return out
